# revision 23
# baseline (speedup 1.0000x reference)
"""Bahdanau attention Trainium2 kernel (8 NeuronCores, batch-sharded).

reference:
    a = k @ Wa.T                    [B, D]
    b = xs @ Wb.T                   [B, S, D]
    w = tanh(a[:, None, :] + b)
    scores = w @ energy             [B, S]
    scores = where(mask, scores, -inf)
    attn = softmax(scores, -1)
    ctx = attn @ xs                 [B, D]
    returns (ctx, attn)

B=32, S=2048, D=1024. 8 cores, 4 batches/core. Single pass over xs:
per 512-row block: PE-transpose xs tiles to [j, s] (fp32r), fp32r matmuls
against resident WbT (bias a folded in with a K=1 ones-matmul), tanh on
ACT (PSUM->SBUF fp32), exact-fp32 scores reduction on DVE
(scalar_tensor_tensor accum vs broadcast energy), block-local softmax
(GPSIMD partition max), partial ctx on PE with exp-weights against the
natural-layout tiles, block recombination + final softmax at the end.
attn is produced in a [128, 64] column layout and re-laid-out on host.
"""

import numpy as np
from contextlib import ExitStack

import concourse.bass as bass
import concourse.tile as tile
from concourse import mybir, bacc
from concourse.bass_utils import run_bass_kernel_spmd
from concourse.masks import make_identity

B, S, D = 32, 2048, 1024
NCORES = 8
BPC = B // NCORES          # batches per core = 4
NBLK = S // 512            # 4 s-blocks of 512 per batch
F32 = mybir.dt.float32
F32R = mybir.dt.float32r
AF = mybir.ActivationFunctionType
ALU = mybir.AluOpType


def build(reps: int = 1, stage: str = "full"):
    """Build the per-core program. reps>1 wraps the main loop in a HW For_i
    (identical work each iteration) for wall-clock slope timing.
    stage: debug knob — "prep", "main", or "full"."""
    nc = bacc.Bacc(None, target_bir_lowering=False, debug=False)

    xs_d = nc.dram_tensor("xs", [BPC, S, D], F32, kind="ExternalInput").ap()
    k_d = nc.dram_tensor("k", [BPC, D], F32, kind="ExternalInput").ap()
    maskc_d = nc.dram_tensor("mask_col", [128, 64], F32, kind="ExternalInput").ap()
    masku_d = nc.dram_tensor("mask_col_u8", [128, 64], mybir.dt.uint8,
                             kind="ExternalInput").ap()
    wa_d = nc.dram_tensor("Wa", [D, D], F32, kind="ExternalInput").ap()
    wb_d = nc.dram_tensor("Wb", [D, D], F32, kind="ExternalInput").ap()
    en_d = nc.dram_tensor("energy", [D], F32, kind="ExternalInput").ap()
    ctx_d = nc.dram_tensor("ctx", [BPC, D], F32, kind="ExternalOutput").ap()
    attnc_d = nc.dram_tensor("attn_col", [128, 64], F32, kind="ExternalOutput").ap()
    pctx_dram = nc.dram_tensor("pctx_scratch", [BPC * NBLK, D], F32).ap()

    with tile.TileContext(nc) as tc, ExitStack() as ctx:
        const = ctx.enter_context(tc.tile_pool(name="const", bufs=1))
        wbt_p = ctx.enter_context(tc.tile_pool(name="wbt", bufs=1))
        xnat_p = ctx.enter_context(tc.tile_pool(name="xnat", bufs=12))
        xst_p = ctx.enter_context(tc.tile_pool(name="xst", bufs=2))
        t_p = ctx.enter_context(tc.tile_pool(name="tp", bufs=4))
        scr_p = ctx.enter_context(tc.tile_pool(name="scr", bufs=2))
        small = ctx.enter_context(tc.tile_pool(name="small", bufs=4))
        ctmp_p = ctx.enter_context(tc.tile_pool(name="ctmp", bufs=2))
        ps_tr = ctx.enter_context(tc.tile_pool(name="pstr", bufs=2, space="PSUM"))
        ps_mb = ctx.enter_context(tc.tile_pool(name="psmb", bufs=2, space="PSUM"))
        ps_cx = ctx.enter_context(tc.tile_pool(name="pscx", bufs=1, space="PSUM"))

        # ---------------- constants ----------------
        ident_f = const.tile([128, 128], F32)
        make_identity(nc, ident_f)
        ident_r = const.tile([128, 128], F32R)
        nc.vector.tensor_copy(ident_r[:], ident_f[:])
        ones_f = const.tile([1, 128], F32)
        nc.vector.memset(ones_f[:], 1.0)
        ones_r = const.tile([1, 128], F32R)
        nc.vector.tensor_copy(ones_r[:], ones_f[:])
        neg_big = const.tile([128, 16], F32)
        nc.vector.memset(neg_big[:], -1e30)

        e_bc = const.tile([128, D], F32)
        e_src = bass.AP(tensor=en_d.tensor, offset=en_d.offset, ap=[[0, 128], [1, D]])
        nc.sync.dma_start(out=e_bc[:], in_=e_src)
        mask_col = const.tile([128, 64], F32)
        nc.sync.dma_start(out=mask_col[:], in_=maskc_d[:])
        mask_u8 = const.tile([128, 64], mybir.dt.uint8)
        nc.sync.dma_start(out=mask_u8[:], in_=masku_d[:])

        scores_col = const.tile([128, 64], F32)
        m_col = const.tile([128, 16], F32)
        attn_col = const.tile([128, 64], F32)
        wbt = wbt_p.tile([128, 8, D], F32R)

        # ---------------- prep: WbT ----------------
        # WbT[j, o] = Wb[o, j]; o-groups of 4x128 share one PSUM bank.
        for og in range(2):
            wbn = []
            for oc4 in range(4):
                o0 = (og * 4 + oc4) * 128
                wn = xnat_p.tile([128, D], F32R, tag="xn")
                nc.sync.dma_start(out=wn[:], in_=wb_d[o0:o0 + 128, :].bitcast(F32R))
                wbn.append(wn)
            for jc in range(8):
                ptr = ps_tr.tile([128, 4, 128], F32R, tag="tr")
                for oc4 in range(4):
                    nc.tensor.transpose(
                        ptr[:, oc4, :], wbn[oc4][:, jc * 128:(jc + 1) * 128], ident_r[:]
                    )
                nc.vector.tensor_copy(
                    wbt[:, jc, og * 512:(og + 1) * 512], ptr[:].rearrange("p a b -> p (a b)")
                )

        # ---------------- prep: a = k @ Wa.T ----------------
        # kT[i, b] = k[b, i]
        kt = const.tile([128, 8, BPC], F32R)
        kn = xnat_p.tile([BPC, D], F32R, tag="xn")
        nc.sync.dma_start(out=kn[:], in_=k_d[:].bitcast(F32R))
        for ic in range(8):
            pk = ps_tr.tile([128, 4, 128], F32R, tag="tr")
            nc.tensor.transpose(
                pk[:, 0, :BPC], kn[:, ic * 128:(ic + 1) * 128], ident_r[:BPC, :BPC]
            )
            nc.vector.tensor_copy(kt[:, ic, :], pk[:, 0, :BPC])

        a4 = const.tile([BPC, D], F32)
        for oh in range(2):
            wat = xst_p.tile([128, 8, 512], F32R, tag="xst")
            wan = []
            for oc4 in range(4):
                o0 = (oh * 4 + oc4) * 128
                wn = xnat_p.tile([128, D], F32R, tag="xn")
                nc.sync.dma_start(out=wn[:], in_=wa_d[o0:o0 + 128, :].bitcast(F32R))
                wan.append(wn)
            for ic in range(8):
                ptr = ps_tr.tile([128, 4, 128], F32R, tag="tr")
                for oc4 in range(4):
                    nc.tensor.transpose(
                        ptr[:, oc4, :], wan[oc4][:, ic * 128:(ic + 1) * 128], ident_r[:]
                    )
                nc.vector.tensor_copy(
                    wat[:, ic, :], ptr[:].rearrange("p a b -> p (a b)")
                )
            pa = ps_mb.tile([128, 512], F32, tag="mb")
            for ic in range(8):
                nc.tensor.matmul(
                    pa[:BPC, :], kt[:, ic, :], wat[:, ic, :],
                    start=(ic == 0), stop=(ic == 7),
                )
            nc.vector.tensor_copy(a4[:, oh * 512:(oh + 1) * 512], pa[:BPC, :])

        # a as a single row on partition 0 (f32r-typed; matmul rounds inputs)
        a_row = const.tile([1, BPC * D], F32R)
        nc.sync.dma_start(out=a_row[:], in_=a4[:].bitcast(F32R))

        # ---------------- main loop ----------------
        def block_body(b, blk):
            slot4 = b * 16 + blk * 4
            bslot = b * NBLK + blk
            s0 = blk * 512
            xn = []
            for sc in range(4):
                t = xnat_p.tile([128, D], F32R, tag="xn")
                nc.sync.dma_start(
                    out=t[:], in_=xs_d[b, s0 + sc * 128:s0 + (sc + 1) * 128, :].bitcast(F32R)
                )
                xn.append(t)
            xt = xst_p.tile([128, 8, 512], F32R, tag="xst")
            for jc2 in range(4):
                ptr = ps_tr.tile([128, 8, 128], F32R, tag="tr")
                for h in range(2):
                    jc = jc2 * 2 + h
                    for sc in range(4):
                        nc.tensor.transpose(
                            ptr[:, h * 4 + sc, :],
                            xn[sc][:, jc * 128:(jc + 1) * 128], ident_r[:]
                        )
                nc.vector.tensor_copy(
                    xt[:, jc2 * 2:jc2 * 2 + 2, :],
                    ptr[:].rearrange("p a b -> p (a b)")
                )
            for sc in range(4):
                tt = t_p.tile([128, D], F32, tag="t")
                for oc in range(2):
                    pmb = ps_mb.tile([128, 512], F32, tag="mb")
                    for jc in range(8):
                        nc.tensor.matmul(
                            pmb[:],
                            xt[:, jc, sc * 128:(sc + 1) * 128],
                            wbt[:, jc, oc * 512:(oc + 1) * 512],
                            start=(jc == 0), stop=False,
                        )
                    nc.tensor.matmul(
                        pmb[:], ones_r[:],
                        a_row[:, b * D + oc * 512: b * D + (oc + 1) * 512],
                        start=False, stop=True,
                    )
                    nc.scalar.activation(
                        tt[:, oc * 512:(oc + 1) * 512], pmb[:], AF.Tanh
                    )
                sct = scr_p.tile([128, D], F32, tag="scr")
                nc.vector.scalar_tensor_tensor(
                    out=sct[:], in0=tt[:], scalar=1.0, in1=e_bc[:],
                    op0=ALU.mult, op1=ALU.mult,
                    accum_out=scores_col[:, slot4 + sc: slot4 + sc + 1],
                )
            # block max (unmasked, safe upper bound), exp, mask
            m_part = small.tile([128, 1], F32, tag="mp")
            nc.vector.tensor_reduce(
                m_part[:], scores_col[:, slot4:slot4 + 4], mybir.AxisListType.X, ALU.max
            )
            nc.gpsimd.partition_all_reduce(
                m_col[:, bslot:bslot + 1], m_part[:], 128, bass.bass_isa.ReduceOp.max
            )
            neg_m = small.tile([128, 1], F32, tag="nm")
            nc.vector.tensor_scalar_mul(neg_m[:], m_col[:, bslot:bslot + 1], -1.0)
            praw = small.tile([128, 4], F32, tag="praw")
            nc.scalar.activation(
                praw[:], scores_col[:, slot4:slot4 + 4], AF.Exp, bias=neg_m[:]
            )
            p_col = small.tile([128, 4], F32R, tag="pcol")
            nc.vector.tensor_mul(p_col[:], praw[:], mask_col[:, slot4:slot4 + 4])
            return (bslot, p_col, xn)

        def ctx_body(pending):
            # partial ctx on PE: pctx = sum_s p[s] * xs[s, :].
            # Emitted one block late so PE never stalls on the softmax chain.
            bslot, p_col, xn = pending
            ctmp = ctmp_p.tile([1, D], F32, tag="ctmp")
            for oc in range(2):
                pcx = ps_cx.tile([1, 512], F32, tag=f"cx{oc}")
                for sc in range(4):
                    nc.tensor.matmul(
                        pcx[:], p_col[:, sc:sc + 1], xn[sc][:, oc * 512:(oc + 1) * 512],
                        start=(sc == 0), stop=(sc == 3),
                    )
                nc.vector.tensor_copy(ctmp[:, oc * 512:(oc + 1) * 512], pcx[:])
            nc.sync.dma_start(out=pctx_dram[bslot, :][None, :], in_=ctmp[:])

        def main_pass():
            pending = None
            for b in range(BPC):
                for blk in range(NBLK):
                    nxt = block_body(b, blk)
                    if pending is not None:
                        ctx_body(pending)
                    pending = nxt
            ctx_body(pending)

        if stage in ("main", "full"):
            if reps == 1:
                main_pass()
            else:
                with tc.For_i(0, reps, 1) as _:
                    main_pass()

        # ---------------- finale ----------------
        for b in range(BPC) if stage == "full" else []:
            b16 = b * 16
            ms = scr_p.tile([128, 16], F32, tag="ms")
            nc.vector.tensor_copy(ms[:], neg_big[:])
            nc.vector.copy_predicated(
                ms[:], mask_u8[:, b16:b16 + 16], scores_col[:, b16:b16 + 16]
            )
            mf_part = small.tile([128, 1], F32, tag="mp")
            nc.vector.tensor_reduce(
                mf_part[:], ms[:], mybir.AxisListType.X, ALU.max
            )
            mf = small.tile([128, 1], F32, tag="mf")
            nc.gpsimd.partition_all_reduce(
                mf[:], mf_part[:], 128, bass.bass_isa.ReduceOp.max
            )
            neg_mf = small.tile([128, 1], F32, tag="nmf")
            nc.vector.tensor_scalar_mul(neg_mf[:], mf[:], -1.0)
            ecol = small.tile([128, 16], F32, tag="ecol")
            nc.scalar.activation(ecol[:], ms[:], AF.Exp, bias=neg_mf[:])
            zs_part = small.tile([128, 1], F32, tag="zp")
            nc.vector.tensor_reduce(
                zs_part[:], ecol[:], mybir.AxisListType.X, ALU.add
            )
            zbc = small.tile([128, 1], F32, tag="zbc")
            nc.gpsimd.partition_all_reduce(
                zbc[:], zs_part[:], 128, bass.bass_isa.ReduceOp.add
            )
            rz = small.tile([128, 1], F32, tag="rz")
            nc.vector.reciprocal(rz[:], zbc[:])
            nc.vector.tensor_scalar_mul(attn_col[:, b16:b16 + 16], ecol[:], rz[:])
            # block weights w[B] = exp(m_blk - m_final) / Z
            wexp = small.tile([128, 4], F32, tag="wexp")
            nc.scalar.activation(
                wexp[:], m_col[:, b * 4:(b + 1) * 4], AF.Exp, bias=neg_mf[:]
            )
            wn = small.tile([128, 4], F32, tag="wn")
            nc.vector.tensor_scalar_mul(wn[:], wexp[:], rz[:])
            w4r = small.tile([4, 1], F32R, tag="w4r")
            nc.sync.dma_start(out=w4r[:], in_=wn[0:1, 0:4].bitcast(F32R))
            pcb = t_p.tile([4, D], F32R, tag="t")
            nc.sync.dma_start(
                out=pcb[:], in_=pctx_dram[b * NBLK:(b + 1) * NBLK, :].bitcast(F32R)
            )
            ctxb = ctmp_p.tile([1, D], F32, tag="ctmp")
            for oc in range(2):
                pcx = ps_cx.tile([1, 512], F32, tag=f"cx{oc}")
                nc.tensor.matmul(
                    pcx[:], w4r[:], pcb[:, oc * 512:(oc + 1) * 512],
                    start=True, stop=True,
                )
                nc.vector.tensor_copy(ctxb[:, oc * 512:(oc + 1) * 512], pcx[:])
            nc.sync.dma_start(out=ctx_d[b, :][None, :], in_=ctxb[:])

        if stage == "full":
            nc.sync.dma_start(out=attnc_d[:], in_=attn_col[:])
        elif stage == "main":
            nc.sync.dma_start(out=attnc_d[:], in_=scores_col[:])
        else:
            nc.sync.dma_start(out=attnc_d[:], in_=wbt[:, 0, :64].bitcast(F32))
    nc.compile()
    return nc


_CACHE = {}


def _get_nc(reps=1):
    if reps not in _CACHE:
        _CACHE[reps] = build(reps)
    return _CACHE[reps]


def _make_in_maps(k, xs, mask, Wa, Wb, energy):
    k = np.ascontiguousarray(k, dtype=np.float32)
    xs = np.ascontiguousarray(xs, dtype=np.float32)
    Wa = np.ascontiguousarray(Wa, dtype=np.float32)
    Wb = np.ascontiguousarray(Wb, dtype=np.float32)
    energy = np.ascontiguousarray(energy, dtype=np.float32)
    mask_f = mask.astype(np.float32)
    in_maps = []
    for c in range(NCORES):
        bs = slice(c * BPC, (c + 1) * BPC)
        # mask_col[p, b*16 + blk*4 + sc] = mask[c*BPC + b, blk*512 + sc*128 + p]
        mc = mask_f[bs].reshape(BPC, NBLK, 4, 128)          # [b, blk, sc, p]
        mask_col = np.ascontiguousarray(
            mc.transpose(3, 0, 1, 2).reshape(128, 64))
        in_maps.append({
            "xs": xs[bs], "k": k[bs], "mask_col": mask_col,
            "mask_col_u8": mask_col.astype(np.uint8),
            "Wa": Wa, "Wb": Wb, "energy": energy,
        })
    return in_maps


def _assemble(results):
    ctx = np.concatenate([r["ctx"] for r in results], axis=0)
    attn_parts = []
    for r in results:
        ac = r["attn_col"]                                   # [128, 64]
        a = ac.reshape(128, BPC, NBLK, 4).transpose(1, 2, 3, 0).reshape(BPC, S)
        attn_parts.append(a)
    attn = np.concatenate(attn_parts, axis=0)
    return ctx, attn


def kernel(k, xs, mask, Wa, Wb, energy):
    nc = _get_nc(1)
    in_maps = _make_in_maps(k, xs, mask, Wa, Wb, energy)
    res = run_bass_kernel_spmd(nc, in_maps, core_ids=list(range(NCORES)))
    return _assemble(res.results)


# revision 31
# speedup vs baseline: 1.3468x; 1.3468x over previous
"""Bahdanau attention Trainium2 kernel (8 NeuronCores, batch-sharded).

reference:
    a = k @ Wa.T                    [B, D]
    b = xs @ Wb.T                   [B, S, D]
    w = tanh(a[:, None, :] + b)
    scores = w @ energy             [B, S]
    scores = where(mask, scores, -inf)
    attn = softmax(scores, -1)
    ctx = attn @ xs                 [B, D]
    returns (ctx, attn)

B=32, S=2048, D=1024. 8 cores, 4 batches/core. Single pass over xs:
per 512-row block: PE-transpose xs tiles to [j, s] (fp32r), fp32r matmuls
against resident WbT (bias a folded in with a K=1 ones-matmul), tanh on
ACT (PSUM->SBUF fp32), exact-fp32 scores reduction on DVE
(scalar_tensor_tensor accum vs broadcast energy), block-local softmax
(GPSIMD partition max), partial ctx on PE with exp-weights against the
natural-layout tiles, block recombination + final softmax at the end.
attn is produced in a [128, 64] column layout and re-laid-out on host.
"""

import numpy as np
from contextlib import ExitStack

import concourse.bass as bass
import concourse.tile as tile
from concourse import mybir, bacc
from concourse.bass_utils import run_bass_kernel_spmd
from concourse.masks import make_identity

B, S, D = 32, 2048, 1024
NCORES = 8
BPC = B // NCORES          # batches per core = 4
NBLK = S // 512            # 4 s-blocks of 512 per batch
F32 = mybir.dt.float32
F32R = mybir.dt.float32r
AF = mybir.ActivationFunctionType
ALU = mybir.AluOpType


def build(reps: int = 1, stage: str = "full", cut: str = ""):
    """Build the per-core program. reps>1 wraps the main loop in a HW For_i
    (identical work each iteration) for wall-clock slope timing.
    stage: debug knob — "prep", "main", or "full".
    cut: timing-ablation knob — comma list of {tr,mm,tanh,stt,ctx}."""
    cuts = set(cut.split(",")) if cut else set()
    nc = bacc.Bacc(None, target_bir_lowering=False, debug=False)

    xs_d = nc.dram_tensor("xs", [BPC, S, D], F32, kind="ExternalInput").ap()
    k_d = nc.dram_tensor("k", [BPC, D], F32, kind="ExternalInput").ap()
    maskc_d = nc.dram_tensor("mask_col", [128, 64], F32, kind="ExternalInput").ap()
    masku_d = nc.dram_tensor("mask_col_u8", [128, 64], mybir.dt.uint8,
                             kind="ExternalInput").ap()
    wa_d = nc.dram_tensor("Wa", [D, D], F32, kind="ExternalInput").ap()
    wb_d = nc.dram_tensor("Wb", [D, D], F32, kind="ExternalInput").ap()
    en_d = nc.dram_tensor("energy", [D], F32, kind="ExternalInput").ap()
    ctx_d = nc.dram_tensor("ctx", [BPC, D], F32, kind="ExternalOutput").ap()
    attnc_d = nc.dram_tensor("attn_col", [128, 64], F32, kind="ExternalOutput").ap()
    pctx_dram = nc.dram_tensor("pctx_scratch", [BPC * NBLK, D], F32).ap()

    with tile.TileContext(nc) as tc, ExitStack() as ctx:
        const = ctx.enter_context(tc.tile_pool(name="const", bufs=1))
        wbt_p = ctx.enter_context(tc.tile_pool(name="wbt", bufs=1))
        xnat_p = ctx.enter_context(tc.tile_pool(name="xnat", bufs=16))
        xst_p = ctx.enter_context(tc.tile_pool(name="xst", bufs=2))
        t_p = ctx.enter_context(tc.tile_pool(name="tp", bufs=4))
        scr_p = ctx.enter_context(tc.tile_pool(name="scr", bufs=2))
        small = ctx.enter_context(tc.tile_pool(name="small", bufs=4))
        ctmp_p = ctx.enter_context(tc.tile_pool(name="ctmp", bufs=2))
        ps_tr = ctx.enter_context(tc.tile_pool(name="pstr", bufs=2, space="PSUM"))
        ps_mb = ctx.enter_context(tc.tile_pool(name="psmb", bufs=2, space="PSUM"))
        ps_cx = ctx.enter_context(tc.tile_pool(name="pscx", bufs=1, space="PSUM"))

        # ---------------- constants ----------------
        ident_f = const.tile([128, 128], F32)
        make_identity(nc, ident_f)
        ident_r = const.tile([128, 128], F32R)
        nc.vector.tensor_copy(ident_r[:], ident_f[:])
        ones_f = const.tile([1, 128], F32)
        nc.vector.memset(ones_f[:], 1.0)
        ones_r = const.tile([1, 128], F32R)
        nc.vector.tensor_copy(ones_r[:], ones_f[:])
        neg_big = const.tile([128, 16], F32)
        nc.vector.memset(neg_big[:], -1e30)

        e_bc = const.tile([128, D], F32)
        e_src = bass.AP(tensor=en_d.tensor, offset=en_d.offset, ap=[[0, 128], [1, D]])
        nc.sync.dma_start(out=e_bc[:], in_=e_src)
        mask_col = const.tile([128, 64], F32)
        nc.sync.dma_start(out=mask_col[:], in_=maskc_d[:])
        mask_u8 = const.tile([128, 64], mybir.dt.uint8)
        nc.sync.dma_start(out=mask_u8[:], in_=masku_d[:])

        scores_col = const.tile([128, 64], F32)
        nc.vector.memset(scores_col[:], 0.0)
        m_col = const.tile([128, 16], F32)
        attn_col = const.tile([128, 64], F32)
        wbt = wbt_p.tile([128, 8, D], F32R)

        # ---------------- prep: WbT ----------------
        # WbT[j, o] = Wb[o, j]; o-groups of 4x128 share one PSUM bank.
        for og in range(2):
            wbn = []
            for oc4 in range(4):
                o0 = (og * 4 + oc4) * 128
                wn = xnat_p.tile([128, D], F32R, tag="xn")
                nc.sync.dma_start(out=wn[:], in_=wb_d[o0:o0 + 128, :].bitcast(F32R))
                wbn.append(wn)
            for jc in range(8):
                ptr = ps_tr.tile([128, 4, 128], F32R, tag="tr")
                for oc4 in range(4):
                    nc.tensor.transpose(
                        ptr[:, oc4, :], wbn[oc4][:, jc * 128:(jc + 1) * 128], ident_r[:]
                    )
                nc.vector.tensor_copy(
                    wbt[:, jc, og * 512:(og + 1) * 512], ptr[:].rearrange("p a b -> p (a b)")
                )

        # ---------------- prep: a = k @ Wa.T ----------------
        # kT[i, b] = k[b, i]
        kt = const.tile([128, 8, BPC], F32R)
        kn = xnat_p.tile([BPC, D], F32R, tag="xn")
        nc.sync.dma_start(out=kn[:], in_=k_d[:].bitcast(F32R))
        for ic in range(8):
            pk = ps_tr.tile([128, 4, 128], F32R, tag="tr")
            nc.tensor.transpose(
                pk[:, 0, :BPC], kn[:, ic * 128:(ic + 1) * 128], ident_r[:BPC, :BPC]
            )
            nc.vector.tensor_copy(kt[:, ic, :], pk[:, 0, :BPC])

        a4 = const.tile([BPC, D], F32)
        for oh in range(2):
            wat = xst_p.tile([128, 8, 512], F32R, tag="xst")
            wan = []
            for oc4 in range(4):
                o0 = (oh * 4 + oc4) * 128
                wn = xnat_p.tile([128, D], F32R, tag="xn")
                nc.sync.dma_start(out=wn[:], in_=wa_d[o0:o0 + 128, :].bitcast(F32R))
                wan.append(wn)
            for ic in range(8):
                ptr = ps_tr.tile([128, 4, 128], F32R, tag="tr")
                for oc4 in range(4):
                    nc.tensor.transpose(
                        ptr[:, oc4, :], wan[oc4][:, ic * 128:(ic + 1) * 128], ident_r[:]
                    )
                nc.vector.tensor_copy(
                    wat[:, ic, :], ptr[:].rearrange("p a b -> p (a b)")
                )
            pa = ps_mb.tile([128, 512], F32, tag="mb")
            for ic in range(8):
                nc.tensor.matmul(
                    pa[:BPC, :], kt[:, ic, :], wat[:, ic, :],
                    start=(ic == 0), stop=(ic == 7),
                )
            nc.vector.tensor_copy(a4[:, oh * 512:(oh + 1) * 512], pa[:BPC, :])

        # a as a single row on partition 0 (f32r-typed; matmul rounds inputs)
        a_row = const.tile([1, BPC * D], F32R)
        nc.sync.dma_start(out=a_row[:], in_=a4[:].bitcast(F32R))

        # ---------------- main loop ----------------
        def block_body(b, blk):
            slot4 = b * 16 + blk * 4
            bslot = b * NBLK + blk
            s0 = blk * 512
            xn = []
            for sc in range(4):
                t = xnat_p.tile([128, D], F32R, tag="xn")
                nc.sync.dma_start(
                    out=t[:], in_=xs_d[b, s0 + sc * 128:s0 + (sc + 1) * 128, :].bitcast(F32R)
                )
                xn.append(t)
            xt = xst_p.tile([128, 8, 512], F32R, tag="xst")
            if "tr" not in cuts:
                for jc2 in range(4):
                    ptr = ps_tr.tile([128, 8, 128], F32R, tag="tr")
                    for h in range(2):
                        jc = jc2 * 2 + h
                        for sc in range(4):
                            nc.tensor.transpose(
                                ptr[:, h * 4 + sc, :],
                                xn[sc][:, jc * 128:(jc + 1) * 128], ident_r[:]
                            )
                    nc.vector.tensor_copy(
                        xt[:, jc2 * 2:jc2 * 2 + 2, :],
                        ptr[:].rearrange("p a b -> p (a b)")
                    )
            for sc in range(4):
                tt = None
                if "tanh" not in cuts:
                    tt = t_p.tile([128, D], F32, tag="t")
                for oc in range(2):
                    pmb = ps_mb.tile([128, 512], F32, tag="mb")
                    if "mm" not in cuts:
                        for jc in range(8):
                            lhs = (wbt[:, jc, sc * 128:(sc + 1) * 128]
                                   if "tr" in cuts else
                                   xt[:, jc, sc * 128:(sc + 1) * 128])
                            nc.tensor.matmul(
                                pmb[:], lhs,
                                wbt[:, jc, oc * 512:(oc + 1) * 512],
                                start=(jc == 0), stop=False,
                            )
                        nc.tensor.matmul(
                            pmb[:], ones_r[:],
                            a_row[:, b * D + oc * 512: b * D + (oc + 1) * 512],
                            start=False, stop=True,
                        )
                    else:
                        nc.tensor.matmul(
                            pmb[:], ones_r[:],
                            a_row[:, b * D + oc * 512: b * D + (oc + 1) * 512],
                            start=True, stop=True,
                        )
                    if tt is not None:
                        nc.scalar.activation(
                            tt[:, oc * 512:(oc + 1) * 512], pmb[:], AF.Tanh
                        )
                if "stt" not in cuts:
                    sct = scr_p.tile([128, D], F32, tag="scr")
                    nc.vector.scalar_tensor_tensor(
                        out=sct[:], in0=tt if tt is not None else e_bc[:],
                        scalar=1.0, in1=e_bc[:],
                        op0=ALU.mult, op1=ALU.mult,
                        accum_out=scores_col[:, slot4 + sc: slot4 + sc + 1],
                    )
            # block max (unmasked, safe upper bound), exp, mask
            m_part = small.tile([128, 1], F32, tag="mp")
            nc.vector.tensor_reduce(
                m_part[:], scores_col[:, slot4:slot4 + 4], mybir.AxisListType.X, ALU.max
            )
            nc.gpsimd.partition_all_reduce(
                m_col[:, bslot:bslot + 1], m_part[:], 128, bass.bass_isa.ReduceOp.max
            )
            neg_m = small.tile([128, 1], F32, tag="nm")
            nc.vector.tensor_scalar_mul(neg_m[:], m_col[:, bslot:bslot + 1], -1.0)
            praw = small.tile([128, 4], F32, tag="praw")
            nc.scalar.activation(
                praw[:], scores_col[:, slot4:slot4 + 4], AF.Exp, bias=neg_m[:]
            )
            p_col = small.tile([128, 4], F32R, tag="pcol")
            nc.vector.tensor_mul(p_col[:], praw[:], mask_col[:, slot4:slot4 + 4])
            return (bslot, p_col, xn)

        def ctx_body(pending):
            # partial ctx on PE: pctx = sum_s p[s] * xs[s, :].
            # Emitted one block late so PE never stalls on the softmax chain.
            bslot, p_col, xn = pending
            ctmp = ctmp_p.tile([1, D], F32, tag="ctmp")
            for oc in range(2):
                pcx = ps_cx.tile([1, 512], F32, tag=f"cx{oc}")
                for sc in range(4):
                    nc.tensor.matmul(
                        pcx[:], p_col[:, sc:sc + 1], xn[sc][:, oc * 512:(oc + 1) * 512],
                        start=(sc == 0), stop=(sc == 3),
                    )
                nc.vector.tensor_copy(ctmp[:, oc * 512:(oc + 1) * 512], pcx[:])
            # ACT-ring HWDGE: keep this off the SP ring so it never blocks
            # the next block's xs loads.
            nc.scalar.dma_start(out=pctx_dram[bslot, :][None, :], in_=ctmp[:])

        def main_pass():
            from collections import deque
            pending = deque()
            for b in range(BPC):
                for blk in range(NBLK):
                    pending.append(block_body(b, blk))
                    if len(pending) > 2:
                        ctx_body(pending.popleft())
            while pending:
                ctx_body(pending.popleft())

        if stage in ("main", "full"):
            if reps == 1:
                main_pass()
            else:
                with tc.For_i(0, reps, 1) as _:
                    main_pass()

        # ---------------- finale ----------------
        for b in range(BPC) if stage == "full" else []:
            b16 = b * 16
            ms = scr_p.tile([128, 16], F32, tag="ms")
            nc.vector.tensor_copy(ms[:], neg_big[:])
            nc.vector.copy_predicated(
                ms[:], mask_u8[:, b16:b16 + 16], scores_col[:, b16:b16 + 16]
            )
            mf_part = small.tile([128, 1], F32, tag="mp")
            nc.vector.tensor_reduce(
                mf_part[:], ms[:], mybir.AxisListType.X, ALU.max
            )
            mf = small.tile([128, 1], F32, tag="mf")
            nc.gpsimd.partition_all_reduce(
                mf[:], mf_part[:], 128, bass.bass_isa.ReduceOp.max
            )
            neg_mf = small.tile([128, 1], F32, tag="nmf")
            nc.vector.tensor_scalar_mul(neg_mf[:], mf[:], -1.0)
            ecol = small.tile([128, 16], F32, tag="ecol")
            nc.scalar.activation(ecol[:], ms[:], AF.Exp, bias=neg_mf[:])
            zs_part = small.tile([128, 1], F32, tag="zp")
            nc.vector.tensor_reduce(
                zs_part[:], ecol[:], mybir.AxisListType.X, ALU.add
            )
            zbc = small.tile([128, 1], F32, tag="zbc")
            nc.gpsimd.partition_all_reduce(
                zbc[:], zs_part[:], 128, bass.bass_isa.ReduceOp.add
            )
            rz = small.tile([128, 1], F32, tag="rz")
            nc.vector.reciprocal(rz[:], zbc[:])
            nc.vector.tensor_scalar_mul(attn_col[:, b16:b16 + 16], ecol[:], rz[:])
            # block weights w[B] = exp(m_blk - m_final) / Z
            wexp = small.tile([128, 4], F32, tag="wexp")
            nc.scalar.activation(
                wexp[:], m_col[:, b * 4:(b + 1) * 4], AF.Exp, bias=neg_mf[:]
            )
            wn = small.tile([128, 4], F32, tag="wn")
            nc.vector.tensor_scalar_mul(wn[:], wexp[:], rz[:])
            w4r = small.tile([4, 1], F32R, tag="w4r")
            nc.sync.dma_start(out=w4r[:], in_=wn[0:1, 0:4].bitcast(F32R))
            pcb = t_p.tile([4, D], F32R, tag="t")
            nc.sync.dma_start(
                out=pcb[:], in_=pctx_dram[b * NBLK:(b + 1) * NBLK, :].bitcast(F32R)
            )
            ctxb = ctmp_p.tile([1, D], F32, tag="ctmp")
            for oc in range(2):
                pcx = ps_cx.tile([1, 512], F32, tag=f"cx{oc}")
                nc.tensor.matmul(
                    pcx[:], w4r[:], pcb[:, oc * 512:(oc + 1) * 512],
                    start=True, stop=True,
                )
                nc.vector.tensor_copy(ctxb[:, oc * 512:(oc + 1) * 512], pcx[:])
            nc.sync.dma_start(out=ctx_d[b, :][None, :], in_=ctxb[:])

        if stage == "full":
            nc.sync.dma_start(out=attnc_d[:], in_=attn_col[:])
        elif stage == "main":
            nc.sync.dma_start(out=attnc_d[:], in_=scores_col[:])
        else:
            nc.sync.dma_start(out=attnc_d[:], in_=wbt[:, 0, :64].bitcast(F32))
    nc.compile()
    return nc


_CACHE = {}


def _get_nc(reps=1):
    if reps not in _CACHE:
        _CACHE[reps] = build(reps)
    return _CACHE[reps]


def _make_in_maps(k, xs, mask, Wa, Wb, energy):
    k = np.ascontiguousarray(k, dtype=np.float32)
    xs = np.ascontiguousarray(xs, dtype=np.float32)
    Wa = np.ascontiguousarray(Wa, dtype=np.float32)
    Wb = np.ascontiguousarray(Wb, dtype=np.float32)
    energy = np.ascontiguousarray(energy, dtype=np.float32)
    mask_f = mask.astype(np.float32)
    in_maps = []
    for c in range(NCORES):
        bs = slice(c * BPC, (c + 1) * BPC)
        # mask_col[p, b*16 + blk*4 + sc] = mask[c*BPC + b, blk*512 + sc*128 + p]
        mc = mask_f[bs].reshape(BPC, NBLK, 4, 128)          # [b, blk, sc, p]
        mask_col = np.ascontiguousarray(
            mc.transpose(3, 0, 1, 2).reshape(128, 64))
        in_maps.append({
            "xs": xs[bs], "k": k[bs], "mask_col": mask_col,
            "mask_col_u8": mask_col.astype(np.uint8),
            "Wa": Wa, "Wb": Wb, "energy": energy,
        })
    return in_maps


def _assemble(results):
    ctx = np.concatenate([r["ctx"] for r in results], axis=0)
    attn_parts = []
    for r in results:
        ac = r["attn_col"]                                   # [128, 64]
        a = ac.reshape(128, BPC, NBLK, 4).transpose(1, 2, 3, 0).reshape(BPC, S)
        attn_parts.append(a)
    attn = np.concatenate(attn_parts, axis=0)
    return ctx, attn


def kernel(k, xs, mask, Wa, Wb, energy):
    nc = _get_nc(1)
    in_maps = _make_in_maps(k, xs, mask, Wa, Wb, energy)
    res = run_bass_kernel_spmd(nc, in_maps, core_ids=list(range(NCORES)))
    return _assemble(res.results)
